# revision 20
# baseline (speedup 1.0000x reference)
"""Trainium2 Bass kernel for the GCNN layer (nn_GCNNLayer_71536975282326).

out = relu( einsum('nd,nde->ne', x, W_pos) + b_pos
            + einsum('nre,nr->ne', einsum('nd,rde->nre', x, W_dep), counts)
            + counts @ b_dep )
with counts[n,r] = #edges (token n, type r).

v4 strategy (8 NeuronCores, SPMD, one program) — the problem is HBM-bound
(242 MiB of f32 weights), so the levers are weight bytes and overlap:

  - All weights quantized host-side to fp8 e3m4, mean-centered:
    Wq = round_e3m4((W - c) * 32); the 1/32 is folded into the bf16 moving
    operands, the rank-1 centering correction c*sum_d(x) rides the bias
    matmul.  End-to-end scale-relative error ~3.3e-3 (gate 2e-2) and 4x
    less DMA than f32.
  - Sharding is by OUTPUT e-COLUMN: core k holds e-columns 128k..128k+127
    of EVERY W_pos[n] and W_dep[r] (same total bytes as type-sharding) and
    computes its 128 out_T rows COMPLETELY — no ReduceScatter, no
    AllGather, no cross-core synchronization at all.  v2's collective tail
    (44 us) becomes a 3 us DVE+DMA epilogue.
  - Every weight matmul uses the W 128x128 block as the STATIONARY operand
    (fp8 -> compiler-automatic fast weight load, ~25 ns/block) and a thin
    bf16 moving operand: the token's x column (self, width 1, out column =
    token) or the counts-scaled x (dep, width 150).  Everything accumulates
    in ONE PSUM bank [128 e, 150 tokens].
  - counts*x moving operands: one DVE tensor_mul per 4-type group with
    both inputs as stride-0 broadcast views (x repeated over types, counts
    row repeated over d-chunks); the counts row is replicated across
    partitions once by a GpSimd partition_broadcast.
  - Bias: one K=94 f32 matmul (92 counts rows + 2 centering rows), the
    bank's single start=True writer; b_pos lands in the DVE epilogue.
  - Weights pre-tiled on host so every DMA line is 4-10 KiB contiguous:
    one dma_start per 0.5-1.25 MiB unit = 128 descriptors.
"""

import numpy as np
import ml_dtypes

import concourse.bass as bass
import concourse.tile as tile
from concourse import bacc, mybir
from concourse.bass_utils import run_bass_kernel_spmd

N, D, R = 150, 1024, 92
NCORES = 8
P = 128
DC = D // P            # 8 contraction (d) chunks
SELF_G = 5             # tokens per self DMA unit
SELF_UNITS = N // SELF_G   # 15
DEP_G = 2              # types per dep DMA/compute unit
MUL_G = 4              # types per DVE xs multiply (measured DVE sweet spot)
BCAST_G = 4            # types per partition_broadcast chunk
DEP_UNITS = R // DEP_G     # 23
KAUG = R + 2           # 92 counts rows + 2 centering rows
QS = 32.0
F32 = mybir.dt.float32
BF16 = mybir.dt.bfloat16
F8 = mybir.dt.float8e3

NP_BF16 = ml_dtypes.bfloat16
NP_F8 = ml_dtypes.float8_e3m4

_PROG = None


def _build_program():
    nc = bacc.Bacc("TRN2", target_bir_lowering=False, debug=False, num_devices=NCORES)

    # pre-tiled per-core weight slices (e-columns 128k..128k+127):
    # wpos[p, ((n c)) * 128 + e], wdep[p, ((r c)) * 128 + e]
    wpos = nc.dram_tensor("wpos", [P, N * DC * P], F8, kind="ExternalInput")
    wdep = nc.dram_tensor("wdep", [P, R * DC * P], F8, kind="ExternalInput")
    xtf = nc.dram_tensor("xtf", [P, DC * N], BF16, kind="ExternalInput")
    xtl = nc.dram_tensor("xtl", [P, DC * N], BF16, kind="ExternalInput")  # x^T/QS
    crep1 = nc.dram_tensor("crep1", [1, R * N], BF16, kind="ExternalInput")
    baug = nc.dram_tensor("baug", [KAUG, P], F32, kind="ExternalInput")
    caug = nc.dram_tensor("caug", [KAUG, N], F32, kind="ExternalInput")
    bposT = nc.dram_tensor("bposT", [P, N], F32, kind="ExternalInput")
    out_T = nc.dram_tensor("out_T", [P, N], F32, kind="ExternalOutput")

    with tile.TileContext(nc) as tc:
        with (
            tc.tile_pool(name="constp", bufs=1) as constp,
            tc.tile_pool(name="mainps", bufs=1, space=bass.MemorySpace.PSUM) as mainps,
            tc.tile_pool(name="fin", bufs=2) as fin,
            tc.tile_pool(name="wpre", bufs=1) as wpre,
        ):
            # crep1 + xtf ride gpsimd's DIRECT2D path: on-engine transfers
            # complete at ~7-8us, while HWDGE completion semaphores (all 16
            # SDMA engines must acknowledge) fire only at ~13-16us under the
            # concurrent weight traffic -- this chain gates the dep stream
            crep1_t0 = constp.tile([1, R * N], BF16)
            nc.gpsimd.dma_start(out=crep1_t0[:], in_=crep1[:])
            xtf_t0 = constp.tile([P, DC * N], BF16)
            nc.gpsimd.dma_start(out=xtf_t0[:], in_=xtf[:])
            pre0 = wpre.tile([P, SELF_G * DC * P], F8, tag="pre0", name="wpre0")
            nc.sync.dma_start(out=pre0[:], in_=wpos[:, 0 : SELF_G * DC * P])
            baug_t = constp.tile([KAUG, P], F32)
            nc.sync.dma_start(out=baug_t[:], in_=baug[:])
            caug_t = constp.tile([KAUG, N], F32)
            nc.sync.dma_start(out=caug_t[:], in_=caug[:])
            xtf_t = xtf_t0
            crep1_t = crep1_t0
            bposT_t = constp.tile([P, N], F32)
            nc.scalar.dma_start(out=bposT_t[:], in_=bposT[:])
            # x^T/QS for the self moving operand: derived on DVE, saves a load
            xtl_t = constp.tile([P, DC * N], BF16)
            nc.vector.tensor_scalar_mul(xtl_t[:], xtf_t[:], 1.0 / QS)
            # counts/QS replicated across partitions in per-group chunks so
            # the first dep unit's xs only waits ~1us, not a 20us monolith
            crep_t = constp.tile([P, R * N], BF16)
            for g in range((R + BCAST_G - 1) // BCAST_G):
                hi = min((g + 1) * BCAST_G, R)
                nc.gpsimd.partition_broadcast(
                    crep_t[:, g * BCAST_G * N : hi * N],
                    crep1_t[:, g * BCAST_G * N : hi * N],
                )

            # warm the PE's HAM clock gate during the dead window before the
            # bias operands arrive: ~4us of dummy matmuls on a scratch bank
            wsc = constp.tile([P, 512], BF16)
            nc.vector.memset(wsc[:].bitcast(F32), 0.0)
            wps = mainps.tile([P, 512], F32, name="warm", tag="warm")
            for i in range(36):
                nc.tensor.matmul(
                    wps[:, 0:256], wsc[:, 0:P], wsc[:, 0:256],
                    start=(i == 0), stop=(i == 35),
                )

            acc = mainps.tile([P, 512], F32, name="acc", tag="acc")
            # bias + centering corrections: the bank's single start=True writer
            nc.tensor.matmul(
                acc[:, 0:N], baug_t[:], caug_t[:], start=True, stop=False,
            )

            with (
                tc.tile_pool(name="wpool", bufs=14) as wpool,
                tc.tile_pool(name="xspool", bufs=3) as xspool,
            ):
                # gpsimd is reserved for the partition_broadcast chain; the
                # W stream triggers rotate over the two HWDGE queues
                engs = [nc.sync, nc.scalar]
                xs_tiles = {}

                def self_unit(g, eng, stop_last):
                    if g == 0:
                        wt = pre0
                    else:
                        wt = wpool.tile([P, SELF_G * DC * P], F8, tag="w", name=f"ws{g}")
                        eng.dma_start(
                            out=wt[:],
                            in_=wpos[:, g * SELF_G * DC * P : (g + 1) * SELF_G * DC * P],
                        )
                    for j in range(SELF_G):
                        n = g * SELF_G + j
                        for c in range(DC):
                            nc.tensor.matmul(
                                acc[:, n : n + 1],
                                wt[:, (j * DC + c) * P : (j * DC + c + 1) * P],
                                xtl_t[:, c * N + n : c * N + n + 1],
                                start=False,
                                stop=stop_last and j == SELF_G - 1 and c == DC - 1,
                            )

                def dep_unit(g, eng, stop_last):
                    wt = wpool.tile([P, DEP_G * DC * P], F8, tag="w", name=f"wd{g}")
                    eng.dma_start(
                        out=wt[:],
                        in_=wdep[:, g * DEP_G * DC * P : (g + 1) * DEP_G * DC * P],
                    )
                    # one MUL_G-type xs multiply feeds MUL_G/DEP_G compute
                    # sub-units (big ops amortize the DVE per-instr overhead)
                    b = (g * DEP_G) // MUL_G
                    if b not in xs_tiles:
                        tcnt = min(MUL_G, R - b * MUL_G)
                        xst = xspool.tile([P, MUL_G * DC * N], BF16, tag="xs",
                                          name=f"xs{b}")
                        nc.vector.tensor_mul(
                            xst[:, 0 : tcnt * DC * N].rearrange(
                                "p (t c n) -> p t c n", t=tcnt, c=DC),
                            xtf_t[:, None, :]
                            .rearrange("p t (c n) -> p t c n", c=DC)
                            .broadcast_to([P, tcnt, DC, N]),
                            crep_t[:, b * MUL_G * N : (b * MUL_G + tcnt) * N, None]
                            .rearrange("p (t n) c -> p t c n", t=tcnt)
                            .broadcast_to([P, tcnt, DC, N]),
                        )
                        xs_tiles[b] = xst
                    xst = xs_tiles[b]
                    toff = (g * DEP_G) % MUL_G
                    for t in range(DEP_G):
                        for c in range(DC):
                            nc.tensor.matmul(
                                acc[:, 0:N],
                                wt[:, (t * DC + c) * P : (t * DC + c + 1) * P],
                                xst[:, ((toff + t) * DC + c) * N
                                   : ((toff + t) * DC + c + 1) * N],
                                start=False,
                                stop=stop_last and t == DEP_G - 1 and c == DC - 1,
                            )

                # interleave self (DMA-heavy) and dep (PE+DVE-heavy) units;
                # two self units first (gives the DVE xs pipeline lead time),
                # and a dep unit last so the stop lands on a full-width MM
                sched = [("s", 0), ("s", 1), ("s", 2)]
                si, di = 3, 0
                while si < SELF_UNITS or di < DEP_UNITS - 1:
                    if di >= DEP_UNITS - 1 or (
                        si < SELF_UNITS
                        and (si - 3) * (DEP_UNITS - 1) <= di * (SELF_UNITS - 3)
                    ):
                        sched.append(("s", si)); si += 1
                    else:
                        sched.append(("d", di)); di += 1
                sched.append(("d", DEP_UNITS - 1))

                for u, (kind, idx) in enumerate(sched):
                    eng = engs[u % len(engs)]
                    last = u == len(sched) - 1
                    if kind == "s":
                        self_unit(idx, eng, last)
                    else:
                        dep_unit(idx, eng, last)

            # ---- epilogue: out = relu(acc + b_pos^T), straight to HBM ----
            oc = fin.tile([P, N], F32, tag="oc")
            nc.vector.scalar_tensor_tensor(
                oc[:], acc[:, 0:N], 0.0, bposT_t[:],
                mybir.AluOpType.add, mybir.AluOpType.add,
            )
            nc.vector.tensor_scalar_max(oc[:], oc[:], 0.0)
            nc.sync.dma_start(out=out_T[:], in_=oc[:])

    nc.compile()
    return nc


def _get_program():
    global _PROG
    if _PROG is None:
        _PROG = _build_program()
    return _PROG


def _prepare_in_maps(x, W_pos, b_pos, W_dep, b_dep, edge_token, edge_type):
    x = np.asarray(x, dtype=np.float32)
    W_pos = np.asarray(W_pos, dtype=np.float32)
    b_pos = np.asarray(b_pos, dtype=np.float32)
    W_dep = np.asarray(W_dep, dtype=np.float32)
    b_dep = np.asarray(b_dep, dtype=np.float32)
    edge_token = np.asarray(edge_token)
    edge_type = np.asarray(edge_type)

    counts = np.zeros((N, R), np.float32)
    np.add.at(counts, (edge_token, edge_type), 1.0)

    c_pos = float(W_pos.max() + W_pos.min()) / 2.0
    c_dep = float(W_dep.max() + W_dep.min()) / 2.0
    Wpq = np.clip((W_pos - c_pos) * QS, -15.5, 15.5).astype(NP_F8)   # [N, D, D]
    Wdq = np.clip((W_dep - c_dep) * QS, -15.5, 15.5).astype(NP_F8)   # [R, D, D]

    xb = x.astype(NP_BF16)
    xbf = xb.astype(np.float32)
    xT16 = np.ascontiguousarray(xb.T)
    xtf_np = np.ascontiguousarray(
        xT16.reshape(DC, P, N).transpose(1, 0, 2).reshape(P, DC * N)
    )
    xtl_np = np.ascontiguousarray(
        (xbf.T / QS).astype(NP_BF16).reshape(DC, P, N).transpose(1, 0, 2)
        .reshape(P, DC * N)
    )
    sx = xbf.sum(axis=1)
    csum = counts.sum(axis=1)
    crep1_np = np.ascontiguousarray((counts.T / QS).astype(NP_BF16).reshape(1, R * N))

    in_maps = []
    for k in range(NCORES):
        sl = slice(k * P, (k + 1) * P)
        wpos_k = np.ascontiguousarray(
            Wpq[:, :, sl].reshape(N, DC, P, P).transpose(2, 0, 1, 3)
            .reshape(P, N * DC * P)
        )
        wdep_k = np.ascontiguousarray(
            Wdq[:, :, sl].reshape(R, DC, P, P).transpose(2, 0, 1, 3)
            .reshape(P, R * DC * P)
        )
        baug_k = np.empty((KAUG, P), np.float32)
        baug_k[:R] = b_dep[:, sl]
        baug_k[R] = c_dep
        baug_k[R + 1] = c_pos
        caug_k = np.empty((KAUG, N), np.float32)
        caug_k[:R] = counts.T
        caug_k[R] = sx * csum
        caug_k[R + 1] = sx
        bposT_k = np.ascontiguousarray(b_pos[:, sl].T)

        in_maps.append(
            dict(wpos=wpos_k, wdep=wdep_k, xtf=xtf_np, xtl=xtl_np,
                 crep1=crep1_np, baug=baug_k, caug=caug_k, bposT=bposT_k)
        )
    return in_maps


def _run(in_maps, trace=False):
    nc = _get_program()
    return run_bass_kernel_spmd(nc, in_maps, list(range(NCORES)), trace=trace)


def _assemble(res):
    out_T = np.concatenate([res.results[k]["out_T"] for k in range(NCORES)], axis=0)
    return np.ascontiguousarray(out_T.T)


def kernel(x, W_pos, b_pos, W_dep, b_dep, edge_token, edge_type):
    in_maps = _prepare_in_maps(x, W_pos, b_pos, W_dep, b_dep, edge_token, edge_type)
    res = _run(in_maps, trace=False)
    return _assemble(res)


def kernel_traced(x, W_pos, b_pos, W_dep, b_dep, edge_token, edge_type):
    """Like kernel() but with NTFF profiling; returns (output, BassKernelResults)."""
    in_maps = _prepare_in_maps(x, W_pos, b_pos, W_dep, b_dep, edge_token, edge_type)
    res = _run(in_maps, trace=True)
    return _assemble(res), res


def install_ntff_shim():
    """The agent image's antenv lacks axon_hooks; recreate it from the boot
    module's ctypes NTFF driver so run_bass_kernel_spmd(trace=True) can
    capture a neuron-profile. Test-only; kernel() never needs this."""
    import sys
    import types

    try:
        from antenv.axon_hooks import get_axon_ntff_profile_hook  # noqa: F401
        return
    except ImportError:
        pass
    from trn_agent_boot.trn_boot import _ntff_profile_via_ctypes

    hook = _ntff_profile_via_ctypes("/opt/axon/libaxon_pjrt.so")
    mod = types.ModuleType("antenv.axon_hooks")
    mod._hook = hook
    mod.get_axon_ntff_profile_hook = lambda: mod._hook
    mod.set_axon_ntff_profile_hook = lambda h: setattr(mod, "_hook", h)
    sys.modules["antenv.axon_hooks"] = mod


# revision 22
# speedup vs baseline: 1.0150x; 1.0150x over previous
"""Trainium2 Bass kernel for the GCNN layer (nn_GCNNLayer_71536975282326).

out = relu( einsum('nd,nde->ne', x, W_pos) + b_pos
            + einsum('nre,nr->ne', einsum('nd,rde->nre', x, W_dep), counts)
            + counts @ b_dep )
with counts[n,r] = #edges (token n, type r).

v4 strategy (8 NeuronCores, SPMD, one program) — the problem is HBM-bound
(242 MiB of f32 weights), so the levers are weight bytes and overlap:

  - All weights quantized host-side to fp8 e3m4, mean-centered:
    Wq = round_e3m4((W - c) * 32); the 1/32 is folded into the bf16 moving
    operands, the rank-1 centering correction c*sum_d(x) rides the bias
    matmul.  End-to-end scale-relative error ~3.3e-3 (gate 2e-2) and 4x
    less DMA than f32.
  - Sharding is by OUTPUT e-COLUMN: core k holds e-columns 128k..128k+127
    of EVERY W_pos[n] and W_dep[r] (same total bytes as type-sharding) and
    computes its 128 out_T rows COMPLETELY — no ReduceScatter, no
    AllGather, no cross-core synchronization at all.  v2's collective tail
    (44 us) becomes a 3 us DVE+DMA epilogue.
  - Every weight matmul uses the W 128x128 block as the STATIONARY operand
    (fp8 -> compiler-automatic fast weight load, ~25 ns/block) and a thin
    bf16 moving operand: the token's x column (self, width 1, out column =
    token) or the counts-scaled x (dep, width 150).  Everything accumulates
    in ONE PSUM bank [128 e, 150 tokens].
  - counts*x moving operands: one DVE tensor_mul per 4-type group with
    both inputs as stride-0 broadcast views (x repeated over types, counts
    row repeated over d-chunks); the counts row is replicated across
    partitions once by a GpSimd partition_broadcast.
  - Bias: one K=94 f32 matmul (92 counts rows + 2 centering rows), the
    bank's single start=True writer; b_pos lands in the DVE epilogue.
  - Weights pre-tiled on host so every DMA line is 4-10 KiB contiguous:
    one dma_start per 0.5-1.25 MiB unit = 128 descriptors.
"""

import numpy as np
import ml_dtypes

import concourse.bass as bass
import concourse.tile as tile
from concourse import bacc, mybir
from concourse.bass_utils import run_bass_kernel_spmd

N, D, R = 150, 1024, 92
NCORES = 8
P = 128
DC = D // P            # 8 contraction (d) chunks
SELF_G = 5             # tokens per self DMA unit
SELF_UNITS = N // SELF_G   # 15
DEP_G = 2              # types per dep DMA/compute unit
MUL_G = 4              # types per DVE xs multiply (measured DVE sweet spot)
BCAST_G = 4            # types per partition_broadcast chunk
DEP_UNITS = R // DEP_G     # 23
KAUG = R + 2           # 92 counts rows + 2 centering rows
QS = 32.0
F32 = mybir.dt.float32
BF16 = mybir.dt.bfloat16
F8 = mybir.dt.float8e3

NP_BF16 = ml_dtypes.bfloat16
NP_F8 = ml_dtypes.float8_e3m4

_PROG = None


def _build_program():
    nc = bacc.Bacc("TRN2", target_bir_lowering=False, debug=False, num_devices=NCORES)

    # pre-tiled per-core weight slices (e-columns 128k..128k+127):
    # wpos[p, ((n c)) * 128 + e], wdep[p, ((r c)) * 128 + e]
    wpos = nc.dram_tensor("wpos", [P, N * DC * P], F8, kind="ExternalInput")
    wdep = nc.dram_tensor("wdep", [P, R * DC * P], F8, kind="ExternalInput")
    xtf = nc.dram_tensor("xtf", [P, DC * N], BF16, kind="ExternalInput")
    xtl = nc.dram_tensor("xtl", [P, DC * N], BF16, kind="ExternalInput")  # x^T/QS
    crep1 = nc.dram_tensor("crep1", [1, R * N], BF16, kind="ExternalInput")
    baug = nc.dram_tensor("baug", [KAUG, P], F32, kind="ExternalInput")
    caug = nc.dram_tensor("caug", [KAUG, N], F32, kind="ExternalInput")
    bposT = nc.dram_tensor("bposT", [P, N], F32, kind="ExternalInput")
    out_T = nc.dram_tensor("out_T", [P, N], F32, kind="ExternalOutput")

    with tile.TileContext(nc) as tc:
        with (
            tc.tile_pool(name="constp", bufs=1) as constp,
            tc.tile_pool(name="mainps", bufs=1, space=bass.MemorySpace.PSUM) as mainps,
            tc.tile_pool(name="fin", bufs=2) as fin,
            tc.tile_pool(name="wpre", bufs=1) as wpre,
        ):
            # the first self unit's weights lead the sync queue so the PE has
            # stream work the moment the warmup drains
            pre0 = wpre.tile([P, SELF_G * DC * P], F8, tag="pre0", name="wpre0")
            nc.sync.dma_start(out=pre0[:], in_=wpos[:, 0 : SELF_G * DC * P])
            baug_t = constp.tile([KAUG, P], F32)
            nc.sync.dma_start(out=baug_t[:], in_=baug[:])
            caug_t = constp.tile([KAUG, N], F32)
            nc.sync.dma_start(out=caug_t[:], in_=caug[:])
            xtf_t = constp.tile([P, DC * N], BF16)
            nc.scalar.dma_start(out=xtf_t[:], in_=xtf[:])
            crep1_t = constp.tile([1, R * N], BF16)
            nc.scalar.dma_start(out=crep1_t[:], in_=crep1[:])
            bposT_t = constp.tile([P, N], F32)
            nc.scalar.dma_start(out=bposT_t[:], in_=bposT[:])
            # x^T/QS for the self moving operand: derived on DVE, saves a load
            xtl_t = constp.tile([P, DC * N], BF16)
            nc.vector.tensor_scalar_mul(xtl_t[:], xtf_t[:], 1.0 / QS)
            # counts/QS replicated across partitions in per-group chunks so
            # the first dep unit's xs only waits ~1us, not a 20us monolith
            crep_t = constp.tile([P, R * N], BF16)
            for g in range(R // BCAST_G):
                nc.gpsimd.partition_broadcast(
                    crep_t[:, g * BCAST_G * N : (g + 1) * BCAST_G * N],
                    crep1_t[:, g * BCAST_G * N : (g + 1) * BCAST_G * N],
                )

            # warm the PE's HAM clock gate during the dead window before the
            # bias operands arrive: ~4us of dummy matmuls on a scratch bank
            wsc = constp.tile([P, 512], BF16)
            nc.vector.memset(wsc[:].bitcast(F32), 0.0)
            wps = mainps.tile([P, 512], F32, name="warm", tag="warm")
            for i in range(36):
                nc.tensor.matmul(
                    wps[:, 0:256], wsc[:, 0:P], wsc[:, 0:256],
                    start=(i == 0), stop=(i == 35),
                )

            acc = mainps.tile([P, 512], F32, name="acc", tag="acc")
            # bias + centering corrections: the bank's single start=True writer
            nc.tensor.matmul(
                acc[:, 0:N], baug_t[:], caug_t[:], start=True, stop=False,
            )

            with (
                tc.tile_pool(name="wpool", bufs=14) as wpool,
                tc.tile_pool(name="xspool", bufs=3) as xspool,
            ):
                # gpsimd is reserved for the partition_broadcast chain; the
                # W stream triggers rotate over the two HWDGE queues
                engs = [nc.sync, nc.scalar]
                xs_tiles = {}

                def self_unit(g, eng, stop_last):
                    if g == 0:
                        wt = pre0
                    else:
                        wt = wpool.tile([P, SELF_G * DC * P], F8, tag="w", name=f"ws{g}")
                        eng.dma_start(
                            out=wt[:],
                            in_=wpos[:, g * SELF_G * DC * P : (g + 1) * SELF_G * DC * P],
                        )
                    for j in range(SELF_G):
                        n = g * SELF_G + j
                        for c in range(DC):
                            nc.tensor.matmul(
                                acc[:, n : n + 1],
                                wt[:, (j * DC + c) * P : (j * DC + c + 1) * P],
                                xtl_t[:, c * N + n : c * N + n + 1],
                                start=False,
                                stop=stop_last and j == SELF_G - 1 and c == DC - 1,
                            )

                def dep_unit(g, eng, stop_last):
                    wt = wpool.tile([P, DEP_G * DC * P], F8, tag="w", name=f"wd{g}")
                    eng.dma_start(
                        out=wt[:],
                        in_=wdep[:, g * DEP_G * DC * P : (g + 1) * DEP_G * DC * P],
                    )
                    # one 4-type xs multiply feeds two 2-type compute units:
                    # 23 x 2.66us muls (DVE sweet spot) instead of 46 x 2.11us,
                    # so the DVE stops pacing the dep stream tail
                    b = (g * DEP_G) // MUL_G
                    if b not in xs_tiles:
                        xst = xspool.tile([P, MUL_G * DC * N], BF16, tag="xs",
                                          name=f"xs{b}")
                        nc.vector.tensor_mul(
                            xst[:].rearrange("p (t c n) -> p t c n", t=MUL_G, c=DC),
                            xtf_t[:, None, :]
                            .rearrange("p t (c n) -> p t c n", c=DC)
                            .broadcast_to([P, MUL_G, DC, N]),
                            crep_t[:, b * MUL_G * N : (b + 1) * MUL_G * N, None]
                            .rearrange("p (t n) c -> p t c n", t=MUL_G)
                            .broadcast_to([P, MUL_G, DC, N]),
                        )
                        xs_tiles[b] = xst
                    xst = xs_tiles[b]
                    toff = (g * DEP_G) % MUL_G
                    for t in range(DEP_G):
                        for c in range(DC):
                            nc.tensor.matmul(
                                acc[:, 0:N],
                                wt[:, (t * DC + c) * P : (t * DC + c + 1) * P],
                                xst[:, ((toff + t) * DC + c) * N
                                   : ((toff + t) * DC + c + 1) * N],
                                start=False,
                                stop=stop_last and t == DEP_G - 1 and c == DC - 1,
                            )

                # interleave self (DMA-heavy) and dep (PE+DVE-heavy) units;
                # two self units first (gives the DVE xs pipeline lead time),
                # and a dep unit last so the stop lands on a full-width MM
                sched = [("s", 0), ("s", 1), ("s", 2)]
                si, di = 3, 0
                while si < SELF_UNITS or di < DEP_UNITS - 1:
                    if di >= DEP_UNITS - 1 or (
                        si < SELF_UNITS
                        and (si - 3) * (DEP_UNITS - 1) <= di * (SELF_UNITS - 3)
                    ):
                        sched.append(("s", si)); si += 1
                    else:
                        sched.append(("d", di)); di += 1
                sched.append(("d", DEP_UNITS - 1))

                for u, (kind, idx) in enumerate(sched):
                    eng = engs[u % len(engs)]
                    last = u == len(sched) - 1
                    if kind == "s":
                        self_unit(idx, eng, last)
                    else:
                        dep_unit(idx, eng, last)

            # ---- epilogue: out = relu(acc + b_pos^T), straight to HBM ----
            oc = fin.tile([P, N], F32, tag="oc")
            nc.vector.scalar_tensor_tensor(
                oc[:], acc[:, 0:N], 0.0, bposT_t[:],
                mybir.AluOpType.add, mybir.AluOpType.add,
            )
            nc.vector.tensor_scalar_max(oc[:], oc[:], 0.0)
            nc.sync.dma_start(out=out_T[:], in_=oc[:])

    nc.compile()
    return nc


def _get_program():
    global _PROG
    if _PROG is None:
        _PROG = _build_program()
    return _PROG


def _prepare_in_maps(x, W_pos, b_pos, W_dep, b_dep, edge_token, edge_type):
    x = np.asarray(x, dtype=np.float32)
    W_pos = np.asarray(W_pos, dtype=np.float32)
    b_pos = np.asarray(b_pos, dtype=np.float32)
    W_dep = np.asarray(W_dep, dtype=np.float32)
    b_dep = np.asarray(b_dep, dtype=np.float32)
    edge_token = np.asarray(edge_token)
    edge_type = np.asarray(edge_type)

    counts = np.zeros((N, R), np.float32)
    np.add.at(counts, (edge_token, edge_type), 1.0)

    c_pos = float(W_pos.max() + W_pos.min()) / 2.0
    c_dep = float(W_dep.max() + W_dep.min()) / 2.0
    Wpq = np.clip((W_pos - c_pos) * QS, -15.5, 15.5).astype(NP_F8)   # [N, D, D]
    Wdq = np.clip((W_dep - c_dep) * QS, -15.5, 15.5).astype(NP_F8)   # [R, D, D]

    xb = x.astype(NP_BF16)
    xbf = xb.astype(np.float32)
    xT16 = np.ascontiguousarray(xb.T)
    xtf_np = np.ascontiguousarray(
        xT16.reshape(DC, P, N).transpose(1, 0, 2).reshape(P, DC * N)
    )
    xtl_np = np.ascontiguousarray(
        (xbf.T / QS).astype(NP_BF16).reshape(DC, P, N).transpose(1, 0, 2)
        .reshape(P, DC * N)
    )
    sx = xbf.sum(axis=1)
    csum = counts.sum(axis=1)
    crep1_np = np.ascontiguousarray((counts.T / QS).astype(NP_BF16).reshape(1, R * N))

    in_maps = []
    for k in range(NCORES):
        sl = slice(k * P, (k + 1) * P)
        wpos_k = np.ascontiguousarray(
            Wpq[:, :, sl].reshape(N, DC, P, P).transpose(2, 0, 1, 3)
            .reshape(P, N * DC * P)
        )
        wdep_k = np.ascontiguousarray(
            Wdq[:, :, sl].reshape(R, DC, P, P).transpose(2, 0, 1, 3)
            .reshape(P, R * DC * P)
        )
        baug_k = np.empty((KAUG, P), np.float32)
        baug_k[:R] = b_dep[:, sl]
        baug_k[R] = c_dep
        baug_k[R + 1] = c_pos
        caug_k = np.empty((KAUG, N), np.float32)
        caug_k[:R] = counts.T
        caug_k[R] = sx * csum
        caug_k[R + 1] = sx
        bposT_k = np.ascontiguousarray(b_pos[:, sl].T)

        in_maps.append(
            dict(wpos=wpos_k, wdep=wdep_k, xtf=xtf_np, xtl=xtl_np,
                 crep1=crep1_np, baug=baug_k, caug=caug_k, bposT=bposT_k)
        )
    return in_maps


def _run(in_maps, trace=False):
    nc = _get_program()
    return run_bass_kernel_spmd(nc, in_maps, list(range(NCORES)), trace=trace)


def _assemble(res):
    out_T = np.concatenate([res.results[k]["out_T"] for k in range(NCORES)], axis=0)
    return np.ascontiguousarray(out_T.T)


def kernel(x, W_pos, b_pos, W_dep, b_dep, edge_token, edge_type):
    in_maps = _prepare_in_maps(x, W_pos, b_pos, W_dep, b_dep, edge_token, edge_type)
    res = _run(in_maps, trace=False)
    return _assemble(res)


def kernel_traced(x, W_pos, b_pos, W_dep, b_dep, edge_token, edge_type):
    """Like kernel() but with NTFF profiling; returns (output, BassKernelResults)."""
    in_maps = _prepare_in_maps(x, W_pos, b_pos, W_dep, b_dep, edge_token, edge_type)
    res = _run(in_maps, trace=True)
    return _assemble(res), res


def install_ntff_shim():
    """The agent image's antenv lacks axon_hooks; recreate it from the boot
    module's ctypes NTFF driver so run_bass_kernel_spmd(trace=True) can
    capture a neuron-profile. Test-only; kernel() never needs this."""
    import sys
    import types

    try:
        from antenv.axon_hooks import get_axon_ntff_profile_hook  # noqa: F401
        return
    except ImportError:
        pass
    from trn_agent_boot.trn_boot import _ntff_profile_via_ctypes

    hook = _ntff_profile_via_ctypes("/opt/axon/libaxon_pjrt.so")
    mod = types.ModuleType("antenv.axon_hooks")
    mod._hook = hook
    mod.get_axon_ntff_profile_hook = lambda: mod._hook
    mod.set_axon_ntff_profile_hook = lambda h: setattr(mod, "_hook", h)
    sys.modules["antenv.axon_hooks"] = mod
